# revision 27
# baseline (speedup 1.0000x reference)
"""MixedScore MultiHeadAttention Trainium2 kernel (8 NeuronCores).

score = ((q + z1) . (k + z2)) / sqrt(d), z1/z2 = per-(n,m) projections of z.
Fused per (b, n): project z[b,n] with W1/W2, add q/k, multiply, segment-sum
over d (selector matmul), softmax over m, weighted sum with v.  z is read
exactly once (fp16).

V3 engine split, processing rows in PAIRS:
  PE    : exactly 3 MM512/row (W1 proj, W2 proj, selector) - no preloads
  Scalar: a_sb = a_ps + q per row (psum exit, bias fused)
  Vector: b2_sb = b2_ps + kt2 per PAIR ([128,1024] op, amortizes overhead)
          + ~1/3 of products + en-normalize + et copies
  GpSimd: ~2/3 of products (sbuf fp16)
Startup: consts ride the scalar HWDGE ring in parallel with z on the sync
ring; warmup ops heat HAM/exp-table/dispatch during the first DMA wait.

Sharding: 8 cores = (b in {0,1}) x (four 128-row n-chunks).  No collectives.
"""

import sys

sys.path.insert(0, "/opt/trn_rl_repo")

import numpy as np

import concourse.bacc as bacc
import concourse.bass as bass
import concourse.tile as tile
from concourse import mybir
from concourse.bass_utils import run_bass_kernel_spmd

B, H, N, D, E = 2, 8, 512, 16, 128
HD = H * D  # 128
NCHUNK = N // 4  # 128 query rows per core
NB = 16  # query rows per batch (16*8 heads = 128 psum rows)
NBATCH = NCHUNK // NB  # 8
NPAIR = NCHUNK // 2  # 64
F32 = mybir.dt.float32
F16 = mybir.dt.float16

SEL_LAG = 5  # selector matmuls lag the pair pipeline by this many pairs
PROD_LAG = 2  # products lag their pair so their inputs are always ready
TR_LAG = 4  # normalize+transposes deferred this many pairs after exp
DVE_PROD_EVERY = 3  # pairs with p % this == 0 multiply on Vector, rest GpSimd
EXP_BIAS = -6.0  # keeps exp() in fp16 range; softmax shift-invariant
WARM_MM = 28  # warmup matmuls while waiting for first z tiles

_CACHE = {}


def _build_program(nchunk=NCHUNK):
    nc = bacc.Bacc("TRN2", target_bir_lowering=False, debug=False,
                   enable_asserts=True, num_devices=8)

    zt_d = nc.dram_tensor("zt_s", [NBATCH, E, NB, N], F16, kind="ExternalInput")
    qt_d = nc.dram_tensor("qt_s", [HD, nchunk], F32, kind="ExternalInput")
    kt2_d = nc.dram_tensor("kt2_s", [HD, 2, N], F16, kind="ExternalInput")
    vf_d = nc.dram_tensor("vf_s", [N, HD], F16, kind="ExternalInput")
    w1_d = nc.dram_tensor("w1", [E, HD], F16, kind="ExternalInput")
    w2_d = nc.dram_tensor("w2", [E, HD], F16, kind="ExternalInput")
    ssel_d = nc.dram_tensor("ssel", [HD, NB, 128], F16, kind="ExternalInput")
    ident_d = nc.dram_tensor("ident", [128, 128], F16, kind="ExternalInput")
    out_d = nc.dram_tensor("out_s", [nchunk, HD], F32, kind="ExternalOutput")

    with tile.TileContext(nc) as tc:
        with (
            tc.tile_pool(name="consts", bufs=1) as consts,
            tc.tile_pool(name="zq", bufs=8) as zqp,  # batch-0 pair slices
            tc.tile_pool(name="zb", bufs=3) as zbp,  # batches 1.. full tiles
            tc.tile_pool(name="ab", bufs=4) as abp,  # (q+z1) pair tiles
            tc.tile_pool(name="bb", bufs=4) as bbp,  # (k+z2) pair tiles
            tc.tile_pool(name="pp", bufs=7) as ppool,  # product pair tiles
            tc.tile_pool(name="ep", bufs=4) as epool,  # softmax sbuf tiles
            tc.tile_pool(name="psA", bufs=2, space="PSUM") as psA,  # 2 banks
            tc.tile_pool(name="psB", bufs=2, space="PSUM") as psB,  # 4 banks
            tc.tile_pool(name="psS", bufs=1, space="PSUM") as psS,  # 1 bank
            tc.tile_pool(name="psT", bufs=1, space="PSUM") as psT,  # 1 bank
        ):
            # ---- consts on the scalar (ACT) HWDGE ring: runs in parallel
            # with the z stream on the sync (SP) ring ----
            w1_sb = consts.tile([E, HD], F16, tag="w1")
            nc.scalar.dma_start(out=w1_sb[:], in_=w1_d.ap())
            w2_sb = consts.tile([E, HD], F16, tag="w2")
            nc.scalar.dma_start(out=w2_sb[:], in_=w2_d.ap())
            ident_sb = consts.tile([128, 128], F16, tag="ident")
            nc.scalar.dma_start(out=ident_sb[:], in_=ident_d.ap())
            kt2_sb = consts.tile([HD, 2, N], F16, tag="kt2")
            nc.scalar.dma_start(out=kt2_sb[:], in_=kt2_d.ap())
            qt_sb = consts.tile([HD, nchunk], F32, tag="qt")
            nc.scalar.dma_start(out=qt_sb[:], in_=qt_d.ap())
            ssel_sb = consts.tile([HD, NB, 128], F16, tag="ssel")
            nc.scalar.dma_start(out=ssel_sb[:], in_=ssel_d.ap())
            # v in m-partition layout: [m-in-tile, mtile, hd]
            vf_sb = consts.tile([128, 4, HD], F16, tag="vf")
            nc.scalar.dma_start(
                out=vf_sb[:], in_=vf_d.ap().rearrange("(t p) c -> p t c", p=128)
            )

            et_all = consts.tile([128, 4, NBATCH, NB, H], F16, tag="et_all")
            ebias_sb = consts.tile([128, 1], F32, tag="ebias")
            nc.gpsimd.memset(ebias_sb[:], EXP_BIAS)
            # warm feed needs NO DMA: memset-backed so HAM warmup can start
            # during the runtime preamble / first z DMA wait
            warm_sb = consts.tile([128, N], F16, tag="warm")
            nc.gpsimd.memset(warm_sb[:], 0.25)

            # ---- z stream on the sync ring ----
            # batch 0 lands as 8 pair-sized DMAs so row 0 starts ASAP
            zqs = {}
            zbs = {}
            for q8 in range(8):
                zq = zqp.tile([E, 2, N], F16, tag="zq", name=f"zq{q8}")
                nc.sync.dma_start(out=zq[:], in_=zt_d[0, :, 2 * q8:2 * q8 + 2, :])
                zqs[q8] = zq

            def fetch_batch(j):
                if j >= NBATCH:
                    return
                zb = zbp.tile([E, NB, N], F16, tag="zb", name=f"zb{j}")
                nc.sync.dma_start(out=zb[:], in_=zt_d[j])
                zbs[j] = zb

            fetch_batch(1)

            # ---- warmup: heat HAM / exp table / DVE+GPS dispatch while the
            # first z quad is in flight (engines would idle anyway) ----
            dummy = epool.tile([128, 2], F32, tag="rowsum", name="dummy_in")
            nc.gpsimd.memset(dummy[:], 0.0)
            dummy_e = epool.tile([128, 2], F16, tag="dummy_e")
            nc.scalar.activation(
                dummy_e[:], dummy[:], func=mybir.ActivationFunctionType.Exp,
                bias=ebias_sb[:],
            )
            dummy_v = epool.tile([128, 64], F16, tag="dummy_v")
            nc.vector.tensor_mul(dummy_v[:], warm_sb[:, 0:64], warm_sb[:, 64:128])
            dummy_g = epool.tile([128, 64], F16, tag="dummy_g")
            nc.gpsimd.tensor_mul(dummy_g[:], warm_sb[:, 0:64], warm_sb[:, 64:128])
            for w in range(WARM_MM):
                wp = psA.tile([HD, N], F32, tag="a", name=f"warm{w}")
                nc.tensor.matmul(wp[:], warm_sb[:, 0:128], warm_sb[:],
                                 start=True, stop=True)

            # ---- main pair-pipelined loop ----
            score_tiles = {}
            p_tiles = {}
            a_ps_live = {}

            def zcol(g):
                """[E, 512] column block for flat row g"""
                j, nn = g // NB, g % NB
                if j == 0:
                    return zqs[nn // 2][:, nn % 2, :]
                return zbs[j][:, nn, :]

            ab_tiles = {}

            def emit_pair(p):
                g0 = 2 * p
                j, nn0 = g0 // NB, g0 % NB
                if nn0 == 0:
                    fetch_batch(j + 2)
                    score_tiles[j] = psS.tile([128, N], F32, tag="score",
                                              name=f"score{j}")
                # W2 projections first: frees the b2 pipeline earliest and
                # delays reuse of the a-bank ring until after the ACT exits.
                b2_ps = psB.tile([HD, 2, N], F32, tag="b2")
                nc.tensor.matmul(b2_ps[:, 0, :], w2_sb[:], zcol(g0))
                nc.tensor.matmul(b2_ps[:, 1, :], w2_sb[:], zcol(g0 + 1))
                a0_ps = psA.tile([HD, N], F32, tag="a", name=f"a0_{p}")
                nc.tensor.matmul(a0_ps[:], w1_sb[:], zcol(g0))
                a1_ps = psA.tile([HD, N], F32, tag="a", name=f"a1_{p}")
                nc.tensor.matmul(a1_ps[:], w1_sb[:], zcol(g0 + 1))

                # one [128, 1024] DVE add covers k-add + psum exit for 2 rows
                b2_sb = bbp.tile([HD, 2, N], F16, tag="b2_sb")
                nc.vector.tensor_add(b2_sb[:], b2_ps[:], kt2_sb[:])

                a2_sb = abp.tile([HD, 2, N], F16, tag="a2_sb")
                nc.scalar.add(a2_sb[:, 0, :], a0_ps[:], qt_sb[:, g0:g0 + 1])
                nc.scalar.add(a2_sb[:, 1, :], a1_ps[:], qt_sb[:, g0 + 1:g0 + 2])
                ab_tiles[p] = (a2_sb, b2_sb)

            def emit_prod(p):
                """lagged so inputs are ready: no engine-FIFO blocking"""
                a2_sb, b2_sb = ab_tiles.pop(p)
                p2 = ppool.tile([HD, 2, N], F16, tag="p2")
                if p % DVE_PROD_EVERY == 0:
                    nc.vector.tensor_mul(p2[:], a2_sb[:], b2_sb[:])
                else:
                    # per-row ops: row-0 selector can fire halfway through
                    nc.gpsimd.tensor_mul(p2[:, 0, :], a2_sb[:, 0, :],
                                         b2_sb[:, 0, :])
                    nc.gpsimd.tensor_mul(p2[:, 1, :], a2_sb[:, 1, :],
                                         b2_sb[:, 1, :])
                p_tiles[p] = p2

            def emit_sels(p):
                g0 = 2 * p
                j, nn0 = g0 // NB, g0 % NB
                p2 = p_tiles.pop(p)
                nc.tensor.matmul(
                    score_tiles[j][:], ssel_sb[:, nn0, :], p2[:, 0, :],
                    start=(nn0 == 0), stop=False,
                )
                nc.tensor.matmul(
                    score_tiles[j][:], ssel_sb[:, nn0 + 1, :], p2[:, 1, :],
                    start=False, stop=(nn0 + 1 == NB - 1),
                )

            def emit_exp(j):
                score_ps = score_tiles.pop(j)
                e_sb = epool.tile([128, N], F16, tag="e")
                rowsum = epool.tile([128, 1], F32, tag="rowsum")
                nc.scalar.activation(
                    e_sb[:], score_ps[:], func=mybir.ActivationFunctionType.Exp,
                    bias=ebias_sb[:], accum_out=rowsum[:],
                )
                return e_sb, rowsum

            def emit_transpose(j, e_sb, rowsum):
                # deferred normalize: by now exp(j) is long done, so the
                # recip doesn't head-of-line-block the DVE ADD stream
                rinv = epool.tile([128, 1], F32, tag="rinv")
                nc.vector.reciprocal(rinv[:], rowsum[:])
                en_sb = epool.tile([128, N], F16, tag="en")
                nc.vector.tensor_scalar_mul(en_sb[:], e_sb[:], rinv[:])
                et_ps = psT.tile([128, N], F16, tag="et", name=f"et_{j}")
                for t in range(4):
                    nc.tensor.transpose(
                        et_ps[:, t * 128:(t + 1) * 128],
                        en_sb[:, t * 128:(t + 1) * 128],
                        ident_sb[:],
                    )
                src = et_ps[:].rearrange("p (a b c) -> p a b c", b=NB, c=H)
                if j == NBATCH - 1:  # tail: DVE is idle, ACT queue is long
                    nc.vector.tensor_copy(out=et_all[:, :, j, :, :], in_=src)
                else:
                    nc.scalar.copy(out=et_all[:, :, j, :, :], in_=src)

            pending_sm = None  # (j, e_sb, rowsum) awaiting normalize+transpose
            for t in range(NPAIR + SEL_LAG):
                # selectors/softmax first: puts exp ahead of this pair's
                # a-exits in the ACT FIFO (score bank frees sooner)
                ts = t - SEL_LAG
                if ts >= 0:
                    emit_sels(ts)
                    jd, nd = ts // (NB // 2), ts % (NB // 2)
                    if nd == NB // 2 - 1:
                        pending_sm = (jd, *emit_exp(jd))
                if pending_sm is not None:
                    jp = pending_sm[0]
                    if ts >= (jp + 1) * (NB // 2) - 1 + TR_LAG or t == NPAIR + SEL_LAG - 1:
                        emit_transpose(*pending_sm)
                        pending_sm = None
                tq = t - PROD_LAG
                if 0 <= tq < NPAIR:
                    emit_prod(tq)  # SEL_LAG > PROD_LAG covers all pairs
                if t < NPAIR:
                    emit_pair(t)

            # ---- out-stage: et slice (all batches, one head) stationary,
            # v head-slice moving; result lands in final [n, h*16+d] layout.
            # fin[(j n), h*16+d] = sum_m e[m, (j n), h] v[m, h*16+d]
            fin_ps = psA.tile([nchunk, HD], F32, tag="a", name="fin_ps")
            for h in range(H):
                for mt in range(4):
                    nc.tensor.matmul(
                        fin_ps[:, h * D:(h + 1) * D],
                        et_all[:, mt, :, :, h],
                        vf_sb[:, mt, h * D:(h + 1) * D],
                        start=(mt == 0), stop=(mt == 3),
                    )
            fin_sb = epool.tile([nchunk, HD], F32, tag="fin_sb")
            nc.vector.tensor_copy(out=fin_sb[:], in_=fin_ps[:])
            nc.sync.dma_start(out=out_d[:], in_=fin_sb[:])

    nc.compile()
    return nc


def _get_program(nchunk=NCHUNK):
    key = nchunk
    if key not in _CACHE:
        _CACHE[key] = _build_program(nchunk)
    return _CACHE[key]


def _prep_shards(q, k, v, z, Wz1, Wz2):
    q = np.asarray(q, np.float32)
    k = np.asarray(k, np.float32)
    v = np.asarray(v, np.float32)
    z = np.asarray(z, np.float32)
    qt = np.ascontiguousarray(q.transpose(0, 1, 3, 2).reshape(B, HD, N))
    kt = np.ascontiguousarray(k.transpose(0, 1, 3, 2).reshape(B, HD, N)).astype(np.float16)
    kt2 = np.ascontiguousarray(np.stack([kt, kt], axis=2))  # [B, HD, 2, N]
    vf = np.ascontiguousarray(v.transpose(0, 2, 1, 3).reshape(B, N, HD)).astype(np.float16)
    ssel = np.zeros((HD, NB, 128), np.float16)
    for n in range(NB):
        for h in range(H):
            ssel[h * D:(h + 1) * D, n, n * H + h] = 1.0 / np.sqrt(D)
    ident = np.eye(128, dtype=np.float16)
    w1 = np.ascontiguousarray(np.asarray(Wz1, np.float16))
    w2 = np.ascontiguousarray(np.asarray(Wz2, np.float16))

    # z per core: [NBATCH, E, NB, N] fp16, contiguous per batch
    zt16 = z.astype(np.float16)
    in_maps = []
    for c in range(8):
        b, nt = c // 4, c % 4
        n0 = nt * NCHUNK
        zc = zt16[b, n0:n0 + NCHUNK]            # [128n, 512m, 128e]
        zc = zc.transpose(2, 0, 1)              # [E, 128n, 512m]
        zc = zc.reshape(E, NBATCH, NB, N).transpose(1, 0, 2, 3)
        in_maps.append({
            "zt_s": np.ascontiguousarray(zc),
            "qt_s": np.ascontiguousarray(qt[b, :, n0:n0 + NCHUNK]),
            "kt2_s": kt2[b],
            "vf_s": vf[b],
            "w1": w1,
            "w2": w2,
            "ssel": ssel,
            "ident": ident,
        })
    return in_maps


def _run(inputs, trace=False, trace_kwargs=None):
    nc = _get_program()
    in_maps = _prep_shards(inputs["q"], inputs["k"], inputs["v"],
                           inputs["z"], inputs["Wz1"], inputs["Wz2"])
    res = run_bass_kernel_spmd(
        nc, in_maps, core_ids=list(range(8)), trace=trace,
        **(trace_kwargs or {}),
    )
    out = np.empty((B, N, HD), np.float32)
    for c in range(8):
        b, nt = c // 4, c % 4
        out[b, nt * NCHUNK:(nt + 1) * NCHUNK, :] = res.results[c]["out_s"]
    return out, res


def kernel(**inputs):
    out, _ = _run(inputs, trace=False)
    return out


# revision 40
# speedup vs baseline: 1.0187x; 1.0187x over previous
"""MixedScore MultiHeadAttention Trainium2 kernel (8 NeuronCores).

score = ((q + z1) . (k + z2)) / sqrt(d), z1/z2 = per-(n,m) projections of z.
Fused per (b, n): project z[b,n] with W1/W2, add q/k, multiply, segment-sum
over d (selector matmul), softmax over m, weighted sum with v.  z is read
exactly once (fp16).

Engine split, processing rows in PAIRS:
  PE    : exactly 3 MM512/row (W1 proj, W2 proj, selector) - no preloads
  Scalar: a_sb = a_ps + q per row (psum exit, bias fused) + exp + et copies
  Vector: b2_sb = b2_ps + kt2 per PAIR ([128,1024] op, amortizes overhead)
          + 40% of products + en-normalize
  GpSimd: 60% of products (sbuf fp16, split per row so selectors fire early)
Products and selectors are emitted PROD_LAG/SEL_LAG pairs behind the
projections so no instruction ever head-of-line-blocks an engine FIFO.
Startup: consts ride the scalar HWDGE ring in parallel with z on the sync
ring; memset-fed warmup matmuls heat HAM/exp-table during the first DMA
wait and are sized to end right as the first z pair lands.

Sharding: 8 cores = (b in {0,1}) x (four 128-row n-chunks).  No collectives.
"""

import sys

sys.path.insert(0, "/opt/trn_rl_repo")

import numpy as np

import concourse.bacc as bacc
import concourse.bass as bass
import concourse.tile as tile
from concourse import mybir
from concourse.bass_utils import run_bass_kernel_spmd

B, H, N, D, E = 2, 8, 512, 16, 128
HD = H * D  # 128
NCHUNK = N // 4  # 128 query rows per core
NB = 16  # query rows per batch (16*8 heads = 128 psum rows)
NBATCH = NCHUNK // NB  # 8
NPAIR = NCHUNK // 2  # 64
F32 = mybir.dt.float32
F16 = mybir.dt.float16

SEL_LAG = 6  # selector matmuls lag the pair pipeline by this many pairs
PROD_LAG = 2  # products lag their pair so their inputs are always ready
TR_LAG = 2  # transposes deferred this many pairs after softmax
DVE_PROD_EVERY = 3  # pairs with p % this == 0 multiply on Vector, rest GpSimd
EXP_BIAS = -6.0  # keeps exp() in fp16 range; softmax shift-invariant
WARM_MM = 18  # warmup matmuls sized to end right as the first z lands

_CACHE = {}


def _build_program(nchunk=NCHUNK):
    nc = bacc.Bacc("TRN2", target_bir_lowering=False, debug=False,
                   enable_asserts=True, num_devices=8)

    zt_d = nc.dram_tensor("zt_s", [NBATCH, E, NB, N], F16, kind="ExternalInput")
    qt_d = nc.dram_tensor("qt_s", [HD, nchunk], F32, kind="ExternalInput")
    kt2_d = nc.dram_tensor("kt2_s", [HD, 2, N], F16, kind="ExternalInput")
    vf_d = nc.dram_tensor("vf_s", [N, HD], F16, kind="ExternalInput")
    w1_d = nc.dram_tensor("w1", [E, HD], F16, kind="ExternalInput")
    w2_d = nc.dram_tensor("w2", [E, HD], F16, kind="ExternalInput")
    ssel_d = nc.dram_tensor("ssel", [HD, NB, 128], F16, kind="ExternalInput")
    ident_d = nc.dram_tensor("ident", [128, 128], F16, kind="ExternalInput")
    out_d = nc.dram_tensor("out_s", [nchunk, HD], F32, kind="ExternalOutput")

    with tile.TileContext(nc) as tc:
        with (
            tc.tile_pool(name="consts", bufs=1) as consts,
            tc.tile_pool(name="zq", bufs=8) as zqp,  # batch-0 pair slices
            tc.tile_pool(name="zb", bufs=3) as zbp,  # batches 1.. full tiles
            tc.tile_pool(name="ab", bufs=4) as abp,  # (q+z1) pair tiles
            tc.tile_pool(name="bb", bufs=4) as bbp,  # (k+z2) pair tiles
            tc.tile_pool(name="pp", bufs=8) as ppool,  # product pair tiles
            tc.tile_pool(name="ep", bufs=4) as epool,  # softmax sbuf tiles
            tc.tile_pool(name="psA", bufs=2, space="PSUM") as psA,  # 2 banks
            tc.tile_pool(name="psB", bufs=2, space="PSUM") as psB,  # 4 banks
            tc.tile_pool(name="psS", bufs=1, space="PSUM") as psS,  # 1 bank
            tc.tile_pool(name="psT", bufs=1, space="PSUM") as psT,  # 1 bank
        ):
            # ---- consts on the scalar (ACT) HWDGE ring: runs in parallel
            # with the z stream on the sync (SP) ring ----
            w1_sb = consts.tile([E, HD], F16, tag="w1")
            nc.scalar.dma_start(out=w1_sb[:], in_=w1_d.ap())
            w2_sb = consts.tile([E, HD], F16, tag="w2")
            nc.scalar.dma_start(out=w2_sb[:], in_=w2_d.ap())
            ident_sb = consts.tile([128, 128], F16, tag="ident")
            nc.scalar.dma_start(out=ident_sb[:], in_=ident_d.ap())
            kt2_sb = consts.tile([HD, 2, N], F16, tag="kt2")
            nc.scalar.dma_start(out=kt2_sb[:], in_=kt2_d.ap())
            qt_sb = consts.tile([HD, nchunk], F32, tag="qt")
            nc.scalar.dma_start(out=qt_sb[:], in_=qt_d.ap())
            ssel_sb = consts.tile([HD, NB, 128], F16, tag="ssel")
            nc.scalar.dma_start(out=ssel_sb[:], in_=ssel_d.ap())
            # v in m-partition layout: [m-in-tile, mtile, hd]
            vf_sb = consts.tile([128, 4, HD], F16, tag="vf")
            nc.scalar.dma_start(
                out=vf_sb[:], in_=vf_d.ap().rearrange("(t p) c -> p t c", p=128)
            )

            et_all = consts.tile([128, 4, NBATCH, NB, H], F16, tag="et_all")
            ebias_sb = consts.tile([128, 1], F32, tag="ebias")
            nc.gpsimd.memset(ebias_sb[:], EXP_BIAS)
            # warm feed needs NO DMA: memset-backed so HAM warmup can start
            # during the runtime preamble / first z DMA wait
            warm_sb = consts.tile([128, N], F16, tag="warm")
            nc.gpsimd.memset(warm_sb[:], 0.25)

            # ---- z stream on the sync ring ----
            # batch 0 lands as 8 pair-sized DMAs so row 0 starts ASAP
            zqs = {}
            zbs = {}
            for q8 in range(8):
                zq = zqp.tile([E, 2, N], F16, tag="zq", name=f"zq{q8}")
                nc.sync.dma_start(out=zq[:], in_=zt_d[0, :, 2 * q8:2 * q8 + 2, :])
                zqs[q8] = zq

            def fetch_batch(j):
                if j >= NBATCH:
                    return
                zb = zbp.tile([E, NB, N], F16, tag="zb", name=f"zb{j}")
                nc.sync.dma_start(out=zb[:], in_=zt_d[j])
                zbs[j] = zb

            fetch_batch(1)

            # ---- warmup: heat HAM / exp table / DVE+GPS dispatch while the
            # first z quad is in flight (engines would idle anyway) ----
            dummy = epool.tile([128, 2], F32, tag="rowsum", name="dummy_in")
            nc.gpsimd.memset(dummy[:], 0.0)
            dummy_e = epool.tile([128, 2], F16, tag="dummy_e")
            nc.scalar.activation(
                dummy_e[:], dummy[:], func=mybir.ActivationFunctionType.Exp,
                bias=ebias_sb[:],
            )
            dummy_v = epool.tile([128, 64], F16, tag="dummy_v")
            nc.vector.tensor_mul(dummy_v[:], warm_sb[:, 0:64], warm_sb[:, 64:128])
            dummy_g = epool.tile([128, 64], F16, tag="dummy_g")
            nc.gpsimd.tensor_mul(dummy_g[:], warm_sb[:, 0:64], warm_sb[:, 64:128])
            for w in range(WARM_MM):
                wp = psA.tile([HD, N], F32, tag="a", name=f"warm{w}")
                nc.tensor.matmul(wp[:], warm_sb[:, 0:128], warm_sb[:],
                                 start=True, stop=True)

            # ---- main pair-pipelined loop ----
            score_tiles = {}
            p_tiles = {}
            a_ps_live = {}

            def zcol(g):
                """[E, 512] column block for flat row g"""
                j, nn = g // NB, g % NB
                if j == 0:
                    return zqs[nn // 2][:, nn % 2, :]
                return zbs[j][:, nn, :]

            ab_tiles = {}

            def emit_pair(p):
                g0 = 2 * p
                j, nn0 = g0 // NB, g0 % NB
                if nn0 == 0:
                    fetch_batch(j + 2)
                    score_tiles[j] = psS.tile([128, N], F32, tag="score",
                                              name=f"score{j}")
                # W2 projections first: frees the b2 pipeline earliest and
                # delays reuse of the a-bank ring until after the ACT exits.
                b2_ps = psB.tile([HD, 2, N], F32, tag="b2")
                nc.tensor.matmul(b2_ps[:, 0, :], w2_sb[:], zcol(g0))
                nc.tensor.matmul(b2_ps[:, 1, :], w2_sb[:], zcol(g0 + 1))
                a0_ps = psA.tile([HD, N], F32, tag="a", name=f"a0_{p}")
                nc.tensor.matmul(a0_ps[:], w1_sb[:], zcol(g0))
                a1_ps = psA.tile([HD, N], F32, tag="a", name=f"a1_{p}")
                nc.tensor.matmul(a1_ps[:], w1_sb[:], zcol(g0 + 1))

                # one [128, 1024] DVE add covers k-add + psum exit for 2 rows
                b2_sb = bbp.tile([HD, 2, N], F16, tag="b2_sb")
                nc.vector.tensor_add(b2_sb[:], b2_ps[:], kt2_sb[:])

                a2_sb = abp.tile([HD, 2, N], F16, tag="a2_sb")
                nc.scalar.add(a2_sb[:, 0, :], a0_ps[:], qt_sb[:, g0:g0 + 1])
                nc.scalar.add(a2_sb[:, 1, :], a1_ps[:], qt_sb[:, g0 + 1:g0 + 2])
                ab_tiles[p] = (a2_sb, b2_sb)

            def emit_prod(p):
                """lagged so inputs are ready: no engine-FIFO blocking"""
                a2_sb, b2_sb = ab_tiles.pop(p)
                p2 = ppool.tile([HD, 2, N], F16, tag="p2")
                if p % 5 in (0, 2):  # 40% of products on Vector, 60% GpSimd
                    nc.vector.tensor_mul(p2[:], a2_sb[:], b2_sb[:])
                else:
                    # per-row ops: row-0 selector can fire halfway through
                    nc.gpsimd.tensor_mul(p2[:, 0, :], a2_sb[:, 0, :],
                                         b2_sb[:, 0, :])
                    nc.gpsimd.tensor_mul(p2[:, 1, :], a2_sb[:, 1, :],
                                         b2_sb[:, 1, :])
                p_tiles[p] = p2

            def emit_sels(p):
                g0 = 2 * p
                j, nn0 = g0 // NB, g0 % NB
                p2 = p_tiles.pop(p)
                nc.tensor.matmul(
                    score_tiles[j][:], ssel_sb[:, nn0, :], p2[:, 0, :],
                    start=(nn0 == 0), stop=False,
                )
                nc.tensor.matmul(
                    score_tiles[j][:], ssel_sb[:, nn0 + 1, :], p2[:, 1, :],
                    start=False, stop=(nn0 + 1 == NB - 1),
                )

            def emit_softmax(j):
                score_ps = score_tiles.pop(j)
                e_sb = epool.tile([128, N], F16, tag="e")
                rowsum = epool.tile([128, 1], F32, tag="rowsum")
                nc.scalar.activation(
                    e_sb[:], score_ps[:], func=mybir.ActivationFunctionType.Exp,
                    bias=ebias_sb[:], accum_out=rowsum[:],
                )
                rinv = epool.tile([128, 1], F32, tag="rinv")
                nc.vector.reciprocal(rinv[:], rowsum[:])
                en_sb = epool.tile([128, N], F16, tag="en")
                nc.vector.tensor_scalar_mul(en_sb[:], e_sb[:], rinv[:])
                return en_sb

            def emit_transpose(j, en_sb):
                et_ps = psT.tile([128, N], F16, tag="et", name=f"et_{j}")
                for t in range(4):
                    nc.tensor.transpose(
                        et_ps[:, t * 128:(t + 1) * 128],
                        en_sb[:, t * 128:(t + 1) * 128],
                        ident_sb[:],
                    )
                src = et_ps[:].rearrange("p (a b c) -> p a b c", b=NB, c=H)
                if j == NBATCH - 1:  # tail: DVE is idle, ACT queue is long
                    nc.vector.tensor_copy(out=et_all[:, :, j, :, :], in_=src)
                else:
                    nc.scalar.copy(out=et_all[:, :, j, :, :], in_=src)

            pending_sm = None  # (j, en_sb) awaiting transpose
            for t in range(NPAIR + SEL_LAG):
                if t < NPAIR:
                    emit_pair(t)
                tq = t - PROD_LAG
                if 0 <= tq < NPAIR:
                    emit_prod(tq)  # SEL_LAG > PROD_LAG covers all pairs
                ts = t - SEL_LAG
                if ts >= 0:
                    emit_sels(ts)
                    jd, nd = ts // (NB // 2), ts % (NB // 2)
                    if nd == NB // 2 - 1:
                        en = emit_softmax(jd)
                        pending_sm = (jd, en)
                if pending_sm is not None:
                    jp = pending_sm[0]
                    if ts >= (jp + 1) * (NB // 2) - 1 + TR_LAG or t == NPAIR + SEL_LAG - 1:
                        emit_transpose(*pending_sm)
                        pending_sm = None

            # ---- out-stage: et slice (all batches, one head) stationary,
            # v head-slice moving; result lands in final [n, h*16+d] layout.
            # fin[(j n), h*16+d] = sum_m e[m, (j n), h] v[m, h*16+d]
            fin_ps = psA.tile([nchunk, HD], F32, tag="a", name="fin_ps")
            for h in range(H):
                for mt in range(4):
                    nc.tensor.matmul(
                        fin_ps[:, h * D:(h + 1) * D],
                        et_all[:, mt, :, :, h],
                        vf_sb[:, mt, h * D:(h + 1) * D],
                        start=(mt == 0), stop=(mt == 3),
                    )
            fin_sb = epool.tile([nchunk, HD], F32, tag="fin_sb")
            nc.vector.tensor_copy(out=fin_sb[:], in_=fin_ps[:])
            nc.sync.dma_start(out=out_d[:], in_=fin_sb[:])

    nc.compile()
    return nc


def _get_program(nchunk=NCHUNK):
    key = nchunk
    if key not in _CACHE:
        _CACHE[key] = _build_program(nchunk)
    return _CACHE[key]


def _prep_shards(q, k, v, z, Wz1, Wz2):
    q = np.asarray(q, np.float32)
    k = np.asarray(k, np.float32)
    v = np.asarray(v, np.float32)
    z = np.asarray(z, np.float32)
    qt = np.ascontiguousarray(q.transpose(0, 1, 3, 2).reshape(B, HD, N))
    kt = np.ascontiguousarray(k.transpose(0, 1, 3, 2).reshape(B, HD, N)).astype(np.float16)
    kt2 = np.ascontiguousarray(np.stack([kt, kt], axis=2))  # [B, HD, 2, N]
    vf = np.ascontiguousarray(v.transpose(0, 2, 1, 3).reshape(B, N, HD)).astype(np.float16)
    ssel = np.zeros((HD, NB, 128), np.float16)
    for n in range(NB):
        for h in range(H):
            ssel[h * D:(h + 1) * D, n, n * H + h] = 1.0 / np.sqrt(D)
    ident = np.eye(128, dtype=np.float16)
    w1 = np.ascontiguousarray(np.asarray(Wz1, np.float16))
    w2 = np.ascontiguousarray(np.asarray(Wz2, np.float16))

    # z per core: [NBATCH, E, NB, N] fp16, contiguous per batch
    zt16 = z.astype(np.float16)
    in_maps = []
    for c in range(8):
        b, nt = c // 4, c % 4
        n0 = nt * NCHUNK
        zc = zt16[b, n0:n0 + NCHUNK]            # [128n, 512m, 128e]
        zc = zc.transpose(2, 0, 1)              # [E, 128n, 512m]
        zc = zc.reshape(E, NBATCH, NB, N).transpose(1, 0, 2, 3)
        in_maps.append({
            "zt_s": np.ascontiguousarray(zc),
            "qt_s": np.ascontiguousarray(qt[b, :, n0:n0 + NCHUNK]),
            "kt2_s": kt2[b],
            "vf_s": vf[b],
            "w1": w1,
            "w2": w2,
            "ssel": ssel,
            "ident": ident,
        })
    return in_maps


def _run(inputs, trace=False, trace_kwargs=None):
    nc = _get_program()
    in_maps = _prep_shards(inputs["q"], inputs["k"], inputs["v"],
                           inputs["z"], inputs["Wz1"], inputs["Wz2"])
    res = run_bass_kernel_spmd(
        nc, in_maps, core_ids=list(range(8)), trace=trace,
        **(trace_kwargs or {}),
    )
    out = np.empty((B, N, HD), np.float32)
    for c in range(8):
        b, nt = c // 4, c % 4
        out[b, nt * NCHUNK:(nt + 1) * NCHUNK, :] = res.results[c]["out_s"]
    return out, res


def kernel(**inputs):
    out, _ = _run(inputs, trace=False)
    return out
